# revision 3
# baseline (speedup 1.0000x reference)
"""Grouped-Query Attention (16 q heads, 4 kv heads, head_dim 128, seq 4096,
hidden 2048) on 8 Trainium2 NeuronCores.

Sharding: sequence-parallel over query tokens (512 per core). Each core
projects q/k/v for its own 512 tokens in bf16; the per-core K^T/V blocks are
AllGathered per kv-head group (4 small collectives, pipelined against
attention on earlier groups), then each core runs full attention for its 512
query rows over all 4096 keys and applies the full output projection,
producing its 512-row slice of the output directly (no reduce needed).

All matmul operands are bf16 (1 cycle/row on the PE array vs 4 for fp32);
accumulation stays fp32 in PSUM. Softmax runs without max-subtraction
(scores are bounded ~|3|): scores are built transposed (S^T[k, q]) per head
pair into one 2-bank PSUM tile and exp'd in a single scalar-engine activation
per pair. The denominator Z = sum_k exp accumulates on the vector engine
(freeing the PE) with a final 128-way partition reduction as one small
matmul; the normalization is a matmul-broadcast of 1/Z folded into the
PSUM->SBUF copy.

Scheduling: initial wk/x DMAs are sliced so the first K-projection matmuls
start after ~0.75 MB of traffic instead of 1.5 MB; the first two output-
projection weight tiles are prefetched during attention so phase 4 starts
the moment the last head pair normalizes; the final k-tile's PSUM
evictions are interleaved per m-tile so the last y DMA issues as early as
possible. Timeline-sim: ~436 us/core (PE busy ~370 us = 84%; the attention
phase is scalar-engine exp-paced at ~294 us, overlapped with PE score/AV
streams).
"""

import numpy as np
import ml_dtypes

import concourse.bass as bass
import concourse.bass_isa as bass_isa
import concourse.bacc as bacc
import concourse.tile as tile
from concourse import mybir
from concourse.bass_utils import run_bass_kernel_spmd

# Problem constants
S = 4096          # sequence length
HID = 2048        # hidden dim
NH = 16           # query heads
NKV = 4           # kv heads
D = 128           # head dim
G = NH // NKV     # q heads per kv head (4)
NC = 8            # cores
SC = S // NC      # tokens per core (512)
P = 128           # partitions
KT = HID // P     # contraction tiles over hidden (16)
INV_NORM = 1.0 / float(np.sqrt(D))

FP = mybir.dt.float32
BF = mybir.dt.bfloat16
BF_NP = ml_dtypes.bfloat16


def build_bass():
    nc = bacc.Bacc(None, num_devices=NC)

    # ---- I/O (all activations/weights pre-cast to bf16 on host) ----
    xTc = nc.declare_dram_parameter("xTc", [HID, SC], BF, isOutput=False)
    # wq pre-tiled on host: [P, NH*KT*D] where col ((o*KT)+h)*D+d holds
    # WqT[h*P+p, o*D+d]
    wq = nc.declare_dram_parameter("wq", [P, NH * KT * D], BF, isOutput=False)
    wk = nc.declare_dram_parameter("wk", [HID, NKV * D], BF, isOutput=False)
    wv = nc.declare_dram_parameter("wv", [HID, NKV * D], BF, isOutput=False)
    # wo pre-tiled on host: [2 halves, 16 k-tiles, 128 o, 1024 m]
    wo = nc.declare_dram_parameter("wo", [2, KT, P, HID // 2], BF, isOutput=False)
    y = nc.declare_dram_parameter("y", [SC, HID], BF, isOutput=True)

    # ---- per-kv-group K^T/V collectives (pipelined against attention) ----
    # kv_loc[g] flat [2, D*SC]: [0] = K^T_g (d-major: d*SC + t),
    #                           [1] = V_g  (t-major: t*D + d)
    kv_loc = [nc.dram_tensor(f"kv_loc{g}", [2, D * SC], BF) for g in range(NKV)]
    kv_gath = [nc.dram_tensor(f"kv_gath{g}", [NC, 2, D * SC], BF,
                              addr_space="Shared") for g in range(NKV)]
    groups = [list(range(NC))]

    with tile.TileContext(nc) as tc:
        with (
            tc.tile_pool(name="const", bufs=1) as const_pool,
            tc.tile_pool(name="qt", bufs=1) as qt_pool,
            tc.tile_pool(name="attn_out", bufs=1) as att_pool,
        ):
            ones_k = const_pool.tile([P, 1], BF)      # Z partition-reduce lhsT
            nc.vector.memset(ones_k[:], 1.0)
            ones_m = const_pool.tile([1, P], BF)      # broadcast lhsT (K=1)
            nc.vector.memset(ones_m[:], 1.0)

            qT_sb = qt_pool.tile([P, NH, SC], BF)           # 2 MB
            attT_sb = att_pool.tile([P, NH, SC], BF)        # 2 MB

            # ---------- Phase 1: local projections ----------
            with (
                tc.tile_pool(name="xw", bufs=1) as xw_pool,
                tc.tile_pool(name="proj_psum", bufs=3, space="PSUM") as pj_psum,
                tc.tile_pool(name="proj_sb", bufs=3) as pj_sb,
                tc.tile_pool(name="wq_sb", bufs=2) as wq_pool,
            ):
                xTc_sb = xw_pool.tile([P, KT, SC], BF)          # 2 MB
                wk_sb = xw_pool.tile([P, KT, NKV * D], BF)      # 1 MB
                wv_sb = xw_pool.tile([P, KT, NKV * D], BF)      # 1 MB
                v_lsb = xw_pool.tile([P, SC // P, NKV * D], BF)  # 0.5 MB
                # issue order matters: group-0 K-proj is chunk-paced by the
                # x stream; interleave small wk/x slices so the first K
                # matmuls start after ~0.75 MB of DMA instead of 1.5 MB
                xr = xTc[:].rearrange("(h p) c -> p h c", p=P)
                wkr = wk[:].rearrange("(h p) c -> p h c", p=P)
                nc.sync.dma_start(out=xTc_sb[:, 0:4, :], in_=xr[:, 0:4, :])
                nc.sync.dma_start(out=wk_sb[:, 0:4, :], in_=wkr[:, 0:4, :])
                nc.sync.dma_start(out=wk_sb[:, 4:16, :], in_=wkr[:, 4:16, :])
                for hq in range(1, 4):
                    nc.sync.dma_start(
                        out=xTc_sb[:, 4 * hq:4 * (hq + 1), :],
                        in_=xr[:, 4 * hq:4 * (hq + 1), :])
                nc.sync.dma_start(
                    out=wv_sb[:], in_=wv[:].rearrange("(h p) c -> p h c", p=P))

                def kproj(g):
                    ps = pj_psum.tile([P, SC], FP, name="kps", tag="kps",
                                      bufs=2)
                    for h in range(KT):
                        nc.tensor.matmul(
                            ps[:],
                            wk_sb[:, h, g * D:(g + 1) * D],
                            xTc_sb[:, h, :],
                            start=(h == 0), stop=(h == KT - 1),
                        )
                    sb = pj_sb.tile([P, SC], BF)
                    nc.vector.tensor_copy(sb[:], ps[:])
                    nc.sync.dma_start(
                        out=kv_loc[g][0].rearrange("(p c) -> p c", p=P),
                        in_=sb[:])

                def vproj(c0, c1):
                    # project V columns [c0:c1) for all 4 token tiles
                    for st in range(SC // P):
                        ps = pj_psum.tile([P, NKV * D], FP, name="vps",
                                          tag="vps", bufs=2)
                        for h in range(KT):
                            nc.tensor.matmul(
                                ps[:, 0:c1 - c0],
                                xTc_sb[:, h, st * P:(st + 1) * P],
                                wv_sb[:, h, c0:c1],
                                start=(h == 0), stop=(h == KT - 1),
                            )
                        nc.vector.tensor_copy(
                            v_lsb[:, st, c0:c1], ps[:, 0:c1 - c0])

                def gather(g):
                    nc.sync.dma_start(
                        out=kv_loc[g][1]
                        .rearrange("(s p d) -> p s d", p=P, s=SC // P),
                        in_=v_lsb[:, :, g * D:(g + 1) * D],
                    )
                    nc.gpsimd.collective_compute(
                        "AllGather", mybir.AluOpType.bypass,
                        replica_groups=groups,
                        ins=[kv_loc[g][:]], outs=[kv_gath[g][:]],
                    )

                # group 0 first: its K + V columns only, then its gather
                # launches while the remaining V columns and groups project
                kproj(0)
                vproj(0, D)
                gather(0)
                vproj(D, NKV * D)
                for g in range(1, NKV):
                    kproj(g)
                    gather(g)

                # q^T: [NH*D, SC] ; lhsT = wq tile [P, D], rhs = xTc tile
                CH = 4  # heads per wq chunk
                for c in range(NH // CH):
                    wqc = wq_pool.tile([P, CH, KT, D], BF)
                    nc.sync.dma_start(
                        out=wqc[:],
                        in_=wq[:, c * CH * KT * D:(c + 1) * CH * KT * D]
                        .rearrange("p (o h d) -> p o h d", o=CH, h=KT),
                    )
                    for ol in range(CH):
                        o = c * CH + ol
                        ps = pj_psum.tile([P, SC], FP)
                        for h in range(KT):
                            nc.tensor.matmul(
                                ps[:], wqc[:, ol, h, :], xTc_sb[:, h, :],
                                start=(h == 0), stop=(h == KT - 1),
                            )
                        nc.vector.tensor_copy(qT_sb[:, o, :], ps[:])

            # ---------- Phase 2+3: attention per kv group ----------
            SK = S // P   # 32 key tiles
            W2 = 2 * SC   # pair width (1024)
            # output-projection weight pool opens before attention so the
            # first two wo tiles stream in during attention (kills the
            # phase-4 entry bubble)
            wo_ctx = tc.tile_pool(name="wo_sb", bufs=3)
            wo_pool = wo_ctx.__enter__()
            wo_pre = []
            for k in range(2):
                wot = wo_pool.tile([P, HID // 2], BF)
                nc.sync.dma_start(out=wot[:], in_=wo[0, k])
                wo_pre.append(wot)
            with (
                tc.tile_pool(name="kv_sb", bufs=2) as kv_pool,
                tc.tile_pool(name="st_psum", bufs=2, space="PSUM") as st_psum,
                tc.tile_pool(name="av_psum", bufs=1, space="PSUM") as av_psum,
                tc.tile_pool(name="nm_psum", bufs=1, space="PSUM") as nm_psum,
                tc.tile_pool(name="p_sb", bufs=4) as p_pool,
                tc.tile_pool(name="z_sb", bufs=2) as zs_pool,
            ):
                # cross-pair software pipelining: pair p's final AV window is
                # issued a few score-tiles into pair p+1 (do_tail), and its
                # normalization (do_norm) a couple more in. The normalize
                # matmuls use a dedicated PSUM bank pool so they never
                # contend with the score stream's slots
                pending_tail = []
                pending_norm = []

                def do_tail():
                    if pending_tail:
                        pending_tail.pop()()

                def do_norm(last=False):
                    if not pending_norm:
                        return
                    h0p, avp, ztp = pending_norm.pop()
                    if last:
                        # final pair: normalize without touching PSUM (the
                        # collectives are done, so the gpsimd queue is free)
                        # so the output-projection PSUM pool can allocate the
                        # instant the last attention matmul retires
                        avu = p_pool.tile([P, W2], BF, name="avu", tag="avu",
                                          bufs=2)
                        nc.vector.tensor_copy(avu[:], avp[:])
                        zfull = zs_pool.tile([P, W2], FP, name="zfull",
                                             tag="zfull", bufs=1)
                        nc.gpsimd.partition_all_reduce(
                            zfull[:], ztp[:], P, bass_isa.ReduceOp.add)
                        rfull = zs_pool.tile([P, W2], FP, name="rfull",
                                             tag="rfull", bufs=1)
                        nc.vector.reciprocal(rfull[:], zfull[:])
                        nc.vector.tensor_mul(
                            attT_sb[:, h0p:h0p + 2, :]
                            .rearrange("p h c -> p (h c)"),
                            avu[:], rfull[:],
                        )
                        return
                    # copy unnormalized AV out of PSUM so the next pair can
                    # claim the bank; cast Z to bf16 for the reduce matmul
                    avu = p_pool.tile([P, W2], BF, name="avu", tag="avu",
                                      bufs=2)
                    nc.vector.tensor_copy(avu[:], avp[:])
                    z16 = zs_pool.tile([P, W2], BF, name="z16", tag="z16",
                                       bufs=2)
                    nc.vector.tensor_copy(z16[:], ztp[:])
                    # Z partition-reduce: both heads into one PSUM bank
                    # (head0 at partition 0, head1 at partition 32)
                    zpn = nm_psum.tile([33, SC], FP, name="zpn", tag="zpn")
                    for hl in range(2):
                        nc.tensor.matmul(
                            zpn[32 * hl:32 * hl + 1, :], ones_k[:],
                            z16[:, hl * SC:(hl + 1) * SC],
                            start=True, stop=True,
                        )
                    for hl in range(2):
                        zr = zs_pool.tile([1, SC], FP, name="zr", tag="zr",
                                          bufs=2)
                        nc.vector.reciprocal(
                            zr[:], zpn[32 * hl:32 * hl + 1, :])
                        zr16 = zs_pool.tile([1, SC], BF, name="zr16",
                                            tag="zr16", bufs=2)
                        nc.vector.tensor_copy(zr16[:], zr[:])
                        bcn = nm_psum.tile([P, SC], FP, name="bcn", tag="bcn")
                        nc.tensor.matmul(
                            bcn[:], ones_m[:], zr16[:],
                            start=True, stop=True,
                        )
                        bcs = zs_pool.tile([P, SC], FP, name="bcs", tag="bcs",
                                           bufs=2)
                        nc.vector.tensor_copy(bcs[:], bcn[:])
                        nc.vector.tensor_mul(
                            attT_sb[:, h0p + hl, :],
                            avu[:, hl * SC:(hl + 1) * SC], bcs[:],
                        )

                for g in range(NKV):
                    kT_g = kv_pool.tile([P, NC, SC], BF, tag="kt")   # 1 MB
                    # first source core's block separately so the first score
                    # matmuls can start while the rest still streams in
                    kr = kv_gath[g][:, 0, :].rearrange("j (p c) -> p j c", p=P)
                    nc.sync.dma_start(out=kT_g[:, 0:1, :], in_=kr[:, 0:1, :])
                    nc.sync.dma_start(out=kT_g[:, 1:NC, :], in_=kr[:, 1:NC, :])
                    v_g = kv_pool.tile([P, NC, 4, D], BF, tag="v")   # 1 MB
                    for j in range(NC):
                        nc.sync.dma_start(
                            out=v_g[:, j],
                            in_=kv_gath[g][j, 1, :]
                            .rearrange("(s p d) -> p s d", p=P, s=4),
                        )

                    for hp in range(G // 2):  # head pairs within group
                        h0 = g * G + 2 * hp
                        av = av_psum.tile([P, W2], FP, name="av", tag="av")
                        # Z accumulator in SBUF fp32 (summed over key tiles on
                        # the DVE; partition-reduced on gpsimd at normalize)
                        z_tot = zs_pool.tile([P, W2], FP, name="z_tot",
                                             tag="z_tot", bufs=2)
                        # software-pipelined: scores issued 2-3 sk ahead of
                        # their AV/Z consumption so the PE never waits on exp
                        pt_q = []

                        def consume4(win, av=av, z_tot=z_tot, v_g=v_g):
                            # 4-way exp-tile pre-add tree on the vector engine
                            # (bf16 2x mode), then fp32 accumulate into z_tot
                            zs01 = zs_pool.tile([P, W2], BF, name="zs01",
                                                tag="zs01", bufs=2)
                            zs23 = zs_pool.tile([P, W2], BF, name="zs23",
                                                tag="zs23", bufs=2)
                            zsum = zs_pool.tile([P, W2], BF, name="zsum",
                                                tag="zsum", bufs=2)
                            nc.vector.tensor_add(zs01[:], win[0][1][:], win[1][1][:])
                            nc.vector.tensor_add(zs23[:], win[2][1][:], win[3][1][:])
                            nc.vector.tensor_add(zsum[:], zs01[:], zs23[:])
                            first = win[0][0]
                            # AV first: depends only on exp, so the PE keeps
                            # streaming while the DVE add-tree produces zsum
                            for psk, pp in win:
                                for hl in range(2):
                                    nc.tensor.matmul(
                                        av[:, hl * SC:(hl + 1) * SC],
                                        v_g[:, psk // 4, psk % 4, :],
                                        pp[:, hl * SC:(hl + 1) * SC],
                                        start=(psk == 0), stop=(psk == SK - 1),
                                    )
                            if first == 0:
                                nc.vector.tensor_copy(z_tot[:], zsum[:])
                            else:
                                nc.vector.tensor_add(z_tot[:], z_tot[:], zsum[:])

                        for sk in range(SK):
                            stp = st_psum.tile([P, W2], FP)
                            kblk = kT_g[:, sk // 4, (sk % 4) * P:(sk % 4 + 1) * P]
                            nc.tensor.matmul(
                                stp[:, 0:SC], kblk, qT_sb[:, h0, :],
                                start=True, stop=True,
                            )
                            nc.tensor.matmul(
                                stp[:, SC:W2], kblk, qT_sb[:, h0 + 1, :],
                                start=True, stop=True,
                            )
                            ptile = p_pool.tile([P, W2], BF, name="ptile",
                                                tag="ptile", bufs=8)
                            nc.scalar.activation(
                                ptile[:], stp[:],
                                mybir.ActivationFunctionType.Exp,
                                scale=INV_NORM,
                            )
                            pt_q.append((sk, ptile))
                            if sk == 0:
                                do_tail()  # previous pair's last AV window
                            elif sk == 2:
                                do_norm()  # previous pair's normalization
                            if len(pt_q) == 6:
                                win = pt_q[:4]
                                del pt_q[:4]
                                consume4(win)
                        tail_win = list(pt_q)
                        pending_tail.append(lambda w=tail_win, c=consume4: c(w))
                        pending_norm.append((h0, av, z_tot))
                do_tail()            # last pair's final AV window
                do_norm(last=True)   # last pair's normalization (PSUM-free)

            # ---------- Phase 4: output projection ----------
            MT = SC // P  # 4 query-row tiles
            with (
                tc.tile_pool(name="y_psum", bufs=8, space="PSUM") as y_psum,
                tc.tile_pool(name="y_sb", bufs=3) as ys_pool,
            ):
                NW = HID // 2 // 512  # 2 moving chunks of 512 per half
                for half in range(2):
                    ps = [[y_psum.tile([P, 512], FP, name="yp", tag="yp")
                           for _ in range(NW)] for _ in range(MT)]

                    def evac(m, half=half, ps=ps):
                        ysb = ys_pool.tile([P, HID // 2], BF)
                        # split the PSUM->SBUF eviction across the scalar and
                        # vector engines so half1's slots free up sooner
                        nc.vector.tensor_copy(ysb[:, 0:512], ps[m][0][:])
                        nc.scalar.activation(
                            ysb[:, 512:1024], ps[m][1][:],
                            mybir.ActivationFunctionType.Copy,
                        )
                        nc.sync.dma_start(
                            out=y[m * P:(m + 1) * P,
                                  half * (HID // 2):(half + 1) * (HID // 2)],
                            in_=ysb[:],
                        )

                    for k in range(KT):
                        if half == 0 and k < 2:
                            wot = wo_pre[k]
                        else:
                            wot = wo_pool.tile([P, HID // 2], BF)
                            nc.sync.dma_start(out=wot[:], in_=wo[half, k])
                        for m in range(MT):
                            for n in range(NW):
                                nc.tensor.matmul(
                                    ps[m][n][:],
                                    attT_sb[:, k, m * P:(m + 1) * P],
                                    wot[:, n * 512:(n + 1) * 512],
                                    start=(k == 0), stop=(k == KT - 1),
                                )
                            if k == KT - 1:
                                # evict each m-tile the moment its last
                                # accumulation retires: the PSUM banks free
                                # up for the next half while the remaining
                                # m-tiles still stream, and the final y DMA
                                # starts ~3 us earlier
                                evac(m)
            wo_ctx.__exit__(None, None, None)
    # bacc lowering: splits multi-sem waits (HW allows 1 wait/instruction),
    # moves matmul waits onto LDWEIGHTS, register alloc.
    nc.compile()
    return nc


_CACHED = {}


def _prep_inputs(x, Wq, Wk, Wv, Wo):
    xs = np.ascontiguousarray(x.reshape(S, HID)).astype(np.float32)
    xT = np.ascontiguousarray(xs.T).astype(BF_NP)        # [HID, S]
    wqT = np.ascontiguousarray(Wq.T).astype(BF_NP)       # [HID, NH*D]
    # wq tiled: [P, NH*KT*D] with col ((o*KT)+h)*D+d = wqT[h*P+p, o*D+d]
    wq_t = np.empty((P, NH * KT * D), BF_NP)
    for o in range(NH):
        for h in range(KT):
            c0 = (o * KT + h) * D
            wq_t[:, c0:c0 + D] = wqT[h * P:(h + 1) * P, o * D:(o + 1) * D]
    wkT = np.ascontiguousarray(Wk.T).astype(BF_NP)       # [HID, NKV*D]
    wvT = np.ascontiguousarray(Wv.T).astype(BF_NP)
    woT = np.ascontiguousarray(Wo.T).astype(BF_NP)       # [HID(o), HID(m)]
    wo_t = np.empty((2, KT, P, HID // 2), BF_NP)
    for half in range(2):
        for k in range(KT):
            wo_t[half, k] = woT[k * P:(k + 1) * P,
                                half * (HID // 2):(half + 1) * (HID // 2)]
    in_maps = []
    for c in range(NC):
        in_maps.append({
            "xTc": np.ascontiguousarray(xT[:, c * SC:(c + 1) * SC]),
            "wq": wq_t, "wk": wkT, "wv": wvT, "wo": wo_t,
        })
    return in_maps


def run(x, Wq, Wk, Wv, Wo, trace=False):
    if "nc" not in _CACHED:
        _CACHED["nc"] = build_bass()
    nc = _CACHED["nc"]
    in_maps = _prep_inputs(x, Wq, Wk, Wv, Wo)
    res = run_bass_kernel_spmd(nc, in_maps, list(range(NC)), trace=trace)
    out = np.concatenate(
        [np.asarray(res.results[c]["y"]).astype(np.float32) for c in range(NC)],
        axis=0)
    return out.reshape(1, S, HID), res


def kernel(x, Wq, Wk, Wv, Wo):
    out, _ = run(np.asarray(x), np.asarray(Wq), np.asarray(Wk),
                 np.asarray(Wv), np.asarray(Wo))
    return out



# revision 5
# speedup vs baseline: 1.0015x; 1.0015x over previous
"""Grouped-Query Attention (16 q heads, 4 kv heads, head_dim 128, seq 4096,
hidden 2048) on 8 Trainium2 NeuronCores.

Sharding: sequence-parallel over query tokens (512 per core). Each core
projects q/k/v for its own 512 tokens in bf16; the per-core K^T/V blocks are
AllGathered per kv-head group (4 small collectives, pipelined against
attention on earlier groups), then each core runs full attention for its 512
query rows over all 4096 keys and applies the full output projection,
producing its 512-row slice of the output directly (no reduce needed).

All matmul operands are bf16 (1 cycle/row on the PE array vs 4 for fp32);
accumulation stays fp32 in PSUM. Softmax runs without max-subtraction
(scores are bounded ~|3|): scores are built transposed (S^T[k, q]) per head
pair into one 2-bank PSUM tile and exp'd in a single scalar-engine activation
per pair. The denominator Z = sum_k exp accumulates on the vector engine with
a gpsimd 128-way partition all-reduce at normalize (PSUM-free, Pool engine is
otherwise idle).

Scheduling: attention is scalar-engine exp-paced (~294 us of ACTIVATE) while
the PE's attention work is only ~270 us, so after a minimal prefix (K/V
projections + all 4 gather launches + the first two q heads) attention
starts immediately and the remaining 12 q-head projections are woven INTO
the attention stream as ~1.9 us "filler" units, 4 per head pair -- exactly
the rate the exp pacing leaves as PE slack, so neither engine starves. The
fillers share a dedicated 2-bank PSUM pool (scores 4 + AV 2 + fillers 2 = 8
banks). The first two output-projection weight tiles prefetch during
attention, and the final k-tile's PSUM evictions interleave per m-tile so
the last y DMA issues as early as possible. Timeline-sim (collective cost
calibrated to the measured ~12 us): ~398 us/core, PE busy 357 us = 90%.
"""

import numpy as np
import ml_dtypes

import concourse.bass as bass
import concourse.bass_isa as bass_isa
import concourse.bacc as bacc
import concourse.tile as tile
from concourse import mybir
from concourse.bass_utils import run_bass_kernel_spmd

# Problem constants
S = 4096          # sequence length
HID = 2048        # hidden dim
NH = 16           # query heads
NKV = 4           # kv heads
D = 128           # head dim
G = NH // NKV     # q heads per kv head (4)
NC = 8            # cores
SC = S // NC      # tokens per core (512)
P = 128           # partitions
KT = HID // P     # contraction tiles over hidden (16)
INV_NORM = 1.0 / float(np.sqrt(D))

FP = mybir.dt.float32
BF = mybir.dt.bfloat16
BF_NP = ml_dtypes.bfloat16


def build_bass():
    nc = bacc.Bacc(None, num_devices=NC)

    # ---- I/O (all activations/weights pre-cast to bf16 on host) ----
    xTc = nc.declare_dram_parameter("xTc", [HID, SC], BF, isOutput=False)
    # wq pre-tiled on host: [P, NH*KT*D] where col ((o*KT)+h)*D+d holds
    # WqT[h*P+p, o*D+d]
    wq = nc.declare_dram_parameter("wq", [P, NH * KT * D], BF, isOutput=False)
    wk = nc.declare_dram_parameter("wk", [HID, NKV * D], BF, isOutput=False)
    wv = nc.declare_dram_parameter("wv", [HID, NKV * D], BF, isOutput=False)
    # wo pre-tiled on host: [2 halves, 16 k-tiles, 128 o, 1024 m]
    wo = nc.declare_dram_parameter("wo", [2, KT, P, HID // 2], BF, isOutput=False)
    y = nc.declare_dram_parameter("y", [SC, HID], BF, isOutput=True)

    # ---- per-kv-group K^T/V collectives (pipelined against attention) ----
    # kv_loc[g] flat [2, D*SC]: [0] = K^T_g (d-major: d*SC + t),
    #                           [1] = V_g  (t-major: t*D + d)
    kv_loc = [nc.dram_tensor(f"kv_loc{g}", [2, D * SC], BF) for g in range(NKV)]
    kv_gath = [nc.dram_tensor(f"kv_gath{g}", [NC, 2, D * SC], BF,
                              addr_space="Shared") for g in range(NKV)]
    groups = [list(range(NC))]

    SK = S // P   # 32 key tiles
    W2 = 2 * SC   # pair width (1024)

    with tile.TileContext(nc) as tc:
        with (
            tc.tile_pool(name="qt", bufs=1) as qt_pool,
            tc.tile_pool(name="attn_out", bufs=1) as att_pool,
            tc.tile_pool(name="wo_sb", bufs=3) as wo_pool,
        ):
            qT_sb = qt_pool.tile([P, NH, SC], BF)           # 2 MB
            attT_sb = att_pool.tile([P, NH, SC], BF)        # 2 MB

            # ---------- Phase 1 pools stay open through attention: the
            # projection work for later head groups runs as fillers inside
            # the attention stream ----------
            with (
                tc.tile_pool(name="xw", bufs=1) as xw_pool,
                tc.tile_pool(name="fill_psum", bufs=2, space="PSUM") as fl_psum,
                tc.tile_pool(name="proj_sb", bufs=2) as pj_sb,
                tc.tile_pool(name="wq_sb", bufs=2) as wq_pool,
            ):
                xTc_sb = xw_pool.tile([P, KT, SC], BF)          # 2 MB
                wk_sb = xw_pool.tile([P, KT, NKV * D], BF)      # 1 MB
                wv_sb = xw_pool.tile([P, KT, NKV * D], BF)      # 1 MB
                v_lsb = xw_pool.tile([P, SC // P, NKV * D], BF)  # 0.5 MB
                # issue order matters: group-0 K-proj is chunk-paced by the
                # x stream; interleave small wk/x slices so the first K
                # matmuls start after ~0.75 MB of DMA instead of 1.5 MB
                xr = xTc[:].rearrange("(h p) c -> p h c", p=P)
                wkr = wk[:].rearrange("(h p) c -> p h c", p=P)
                nc.sync.dma_start(out=xTc_sb[:, 0:4, :], in_=xr[:, 0:4, :])
                nc.sync.dma_start(out=wk_sb[:, 0:4, :], in_=wkr[:, 0:4, :])
                nc.sync.dma_start(out=wk_sb[:, 4:16, :], in_=wkr[:, 4:16, :])
                for hq in range(1, 4):
                    nc.sync.dma_start(
                        out=xTc_sb[:, 4 * hq:4 * (hq + 1), :],
                        in_=xr[:, 4 * hq:4 * (hq + 1), :])
                # group-0 V columns first: the prefix vproj(st, 0, D) only
                # needs cols 0:D, so it starts ~3 us sooner
                wvr = wv[:].rearrange("(h p) c -> p h c", p=P)
                nc.sync.dma_start(out=wv_sb[:, :, 0:D], in_=wvr[:, :, 0:D])
                nc.sync.dma_start(
                    out=wv_sb[:, :, D:NKV * D], in_=wvr[:, :, D:NKV * D])

                def kproj(g):
                    ps = fl_psum.tile([P, SC], FP, name="fill", tag="fill",
                                      bufs=2)
                    for h in range(KT):
                        nc.tensor.matmul(
                            ps[:],
                            wk_sb[:, h, g * D:(g + 1) * D],
                            xTc_sb[:, h, :],
                            start=(h == 0), stop=(h == KT - 1),
                        )
                    sb = pj_sb.tile([P, SC], BF)
                    nc.vector.tensor_copy(sb[:], ps[:])
                    nc.sync.dma_start(
                        out=kv_loc[g][0].rearrange("(p c) -> p c", p=P),
                        in_=sb[:])

                def vproj(st, c0, c1):
                    # project V columns [c0:c1) for token tile st
                    ps = fl_psum.tile([P, SC], FP, name="fill", tag="fill",
                                      bufs=2)
                    for h in range(KT):
                        nc.tensor.matmul(
                            ps[:, 0:c1 - c0],
                            xTc_sb[:, h, st * P:(st + 1) * P],
                            wv_sb[:, h, c0:c1],
                            start=(h == 0), stop=(h == KT - 1),
                        )
                    nc.vector.tensor_copy(
                        v_lsb[:, st, c0:c1], ps[:, 0:c1 - c0])

                def gather(g):
                    nc.sync.dma_start(
                        out=kv_loc[g][1]
                        .rearrange("(s p d) -> p s d", p=P, s=SC // P),
                        in_=v_lsb[:, :, g * D:(g + 1) * D],
                    )
                    nc.gpsimd.collective_compute(
                        "AllGather", mybir.AluOpType.bypass,
                        replica_groups=groups,
                        ins=[kv_loc[g][:]], outs=[kv_gath[g][:]],
                    )

                CH = 2  # heads per wq chunk
                _q_state = {}  # (c, ol) -> psum tile; c -> wq chunk tile

                def qproj_unit(c, ol, hf):
                    # one ~1.9us filler unit: 8 of head (c*CH+ol)'s 16
                    # contraction matmuls. hf=0 opens the PSUM accumulation
                    # (and DMAs the wq chunk on the first unit of the
                    # chunk); hf=1 closes it and evicts to qT_sb
                    if ol == 0 and hf == 0:
                        wqc = wq_pool.tile([P, CH, KT, D], BF)
                        nc.sync.dma_start(
                            out=wqc[:],
                            in_=wq[:, c * CH * KT * D:(c + 1) * CH * KT * D]
                            .rearrange("p (o h d) -> p o h d", o=CH, h=KT),
                        )
                        _q_state[c] = wqc
                    wqc = _q_state[c]
                    if hf == 0:
                        ps = fl_psum.tile([P, SC], FP, name="fill",
                                          tag="fill", bufs=2)
                        _q_state[(c, ol)] = ps
                    ps = _q_state[(c, ol)]
                    for h in range(hf * (KT // 2), (hf + 1) * (KT // 2)):
                        nc.tensor.matmul(
                            ps[:], wqc[:, ol, h, :], xTc_sb[:, h, :],
                            start=(h == 0), stop=(h == KT - 1),
                        )
                    if hf == 1:
                        o = c * CH + ol
                        nc.vector.tensor_copy(qT_sb[:, o, :], ps[:])
                        del _q_state[(c, ol)]

                def qproj(c):
                    for ol in range(CH):
                        for hf in range(2):
                            qproj_unit(c, ol, hf)

                # ---- prefix: all K/V projections and gathers launch up
                # front (the collectives pipeline against attention exactly
                # as before), plus the first two q-head pairs ----
                kproj(0)
                for st in range(SC // P):
                    vproj(st, 0, D)
                gather(0)
                for st in range(SC // P):
                    vproj(st, D, NKV * D)
                for g in range(1, NKV):
                    kproj(g)
                    gather(g)
                qproj(0)   # heads 0-1
                qproj(1)   # heads 2-3

                # ---- filler queue: (needed-by-group, ~1.9us unit) in issue
                # order. Units are woven into the attention stream at 4 per
                # head pair — the rate the scalar engine's exp pacing leaves
                # as PE slack — and a group's prologue force-drains
                # everything it depends on ----
                fillers = []
                for c in range(2, NH // CH):
                    need_by = 1 + (c - 2) // 2
                    for ol in range(CH):
                        for hf in range(2):
                            fillers.append(
                                (need_by,
                                 lambda c=c, ol=ol, hf=hf:
                                 qproj_unit(c, ol, hf)))

                def drain():
                    if fillers:
                        fillers.pop(0)[1]()

                def force_drain(g):
                    while fillers and fillers[0][0] <= g:
                        drain()

                # output-projection weight prefetch: streams in during
                # attention, kills the phase-4 entry bubble
                wo_pre = []
                for k in range(2):
                    wot = wo_pool.tile([P, HID // 2], BF)
                    nc.sync.dma_start(out=wot[:], in_=wo[0, k])
                    wo_pre.append(wot)

                # ---------- Phase 2+3: attention per kv group ----------
                with (
                    tc.tile_pool(name="kv_sb", bufs=2) as kv_pool,
                    tc.tile_pool(name="st_psum", bufs=2, space="PSUM") as st_psum,
                    tc.tile_pool(name="av_psum", bufs=1, space="PSUM") as av_psum,
                    tc.tile_pool(name="p_sb", bufs=4) as p_pool,
                    tc.tile_pool(name="z_sb", bufs=2) as zs_pool,
                ):
                    # cross-pair software pipelining: pair p's final AV window
                    # is issued a few score-tiles into pair p+1 (do_tail), and
                    # its normalization (do_norm) a couple more in
                    pending_tail = []
                    pending_norm = []

                    def do_tail():
                        if pending_tail:
                            pending_tail.pop()()

                    def do_norm():
                        if not pending_norm:
                            return
                        h0p, avp, ztp = pending_norm.pop()
                        # PSUM-free normalize: copy unnormalized AV out of
                        # PSUM so the next pair can claim the bank; Z is
                        # partition-reduced on the (otherwise idle) gpsimd
                        # engine, broadcast to every partition
                        avu = p_pool.tile([P, W2], BF, name="avu", tag="avu",
                                          bufs=2)
                        nc.vector.tensor_copy(avu[:], avp[:])
                        zfull = zs_pool.tile([P, W2], FP, name="zfull",
                                             tag="zfull", bufs=1)
                        nc.gpsimd.partition_all_reduce(
                            zfull[:], ztp[:], P, bass_isa.ReduceOp.add)
                        rfull = zs_pool.tile([P, W2], FP, name="rfull",
                                             tag="rfull", bufs=1)
                        nc.vector.reciprocal(rfull[:], zfull[:])
                        nc.vector.tensor_mul(
                            attT_sb[:, h0p:h0p + 2, :]
                            .rearrange("p h c -> p (h c)"),
                            avu[:], rfull[:],
                        )

                    for g in range(NKV):
                        # everything group g depends on must be issued first
                        force_drain(g)
                        kT_g = kv_pool.tile([P, NC, SC], BF, tag="kt")  # 1 MB
                        # first source core's block separately so the first
                        # score matmuls can start while the rest streams in
                        kr = kv_gath[g][:, 0, :].rearrange(
                            "j (p c) -> p j c", p=P)
                        nc.sync.dma_start(out=kT_g[:, 0:1, :], in_=kr[:, 0:1, :])
                        nc.sync.dma_start(out=kT_g[:, 1:NC, :], in_=kr[:, 1:NC, :])
                        v_g = kv_pool.tile([P, NC, 4, D], BF, tag="v")  # 1 MB
                        for j in range(NC):
                            nc.sync.dma_start(
                                out=v_g[:, j],
                                in_=kv_gath[g][j, 1, :]
                                .rearrange("(s p d) -> p s d", p=P, s=4),
                            )

                        for hp in range(G // 2):  # head pairs within group
                            h0 = g * G + 2 * hp
                            av = av_psum.tile([P, W2], FP, name="av", tag="av")
                            # Z accumulator in SBUF fp32 (summed over key
                            # tiles on the DVE; partition-reduced on gpsimd
                            # at normalize)
                            z_tot = zs_pool.tile([P, W2], FP, name="z_tot",
                                                 tag="z_tot", bufs=2)
                            # software-pipelined: scores issued 2-3 sk ahead
                            # of their AV/Z consumption so the PE never waits
                            # on exp
                            pt_q = []

                            def consume4(win, av=av, z_tot=z_tot, v_g=v_g):
                                # 4-way exp-tile pre-add tree on the vector
                                # engine (bf16 2x mode), then fp32 accumulate
                                # into z_tot
                                zs01 = zs_pool.tile([P, W2], BF, name="zs01",
                                                    tag="zs01", bufs=2)
                                zs23 = zs_pool.tile([P, W2], BF, name="zs23",
                                                    tag="zs23", bufs=2)
                                zsum = zs_pool.tile([P, W2], BF, name="zsum",
                                                    tag="zsum", bufs=2)
                                nc.vector.tensor_add(
                                    zs01[:], win[0][1][:], win[1][1][:])
                                nc.vector.tensor_add(
                                    zs23[:], win[2][1][:], win[3][1][:])
                                nc.vector.tensor_add(zsum[:], zs01[:], zs23[:])
                                first = win[0][0]
                                # AV first: depends only on exp, so the PE
                                # keeps streaming while the DVE add-tree
                                # produces zsum
                                for psk, pp in win:
                                    for hl in range(2):
                                        nc.tensor.matmul(
                                            av[:, hl * SC:(hl + 1) * SC],
                                            v_g[:, psk // 4, psk % 4, :],
                                            pp[:, hl * SC:(hl + 1) * SC],
                                            start=(psk == 0),
                                            stop=(psk == SK - 1),
                                        )
                                if first == 0:
                                    nc.vector.tensor_copy(z_tot[:], zsum[:])
                                else:
                                    nc.vector.tensor_add(
                                        z_tot[:], z_tot[:], zsum[:])

                            for sk in range(SK):
                                stp = st_psum.tile([P, W2], FP)
                                kblk = kT_g[:, sk // 4,
                                            (sk % 4) * P:(sk % 4 + 1) * P]
                                nc.tensor.matmul(
                                    stp[:, 0:SC], kblk, qT_sb[:, h0, :],
                                    start=True, stop=True,
                                )
                                nc.tensor.matmul(
                                    stp[:, SC:W2], kblk, qT_sb[:, h0 + 1, :],
                                    start=True, stop=True,
                                )
                                ptile = p_pool.tile([P, W2], BF, name="ptile",
                                                    tag="ptile", bufs=8)
                                nc.scalar.activation(
                                    ptile[:], stp[:],
                                    mybir.ActivationFunctionType.Exp,
                                    scale=INV_NORM,
                                )
                                pt_q.append((sk, ptile))
                                if sk == 0:
                                    do_tail()  # prev pair's last AV window
                                elif sk == 2:
                                    do_norm()  # prev pair's normalization
                                elif sk in (4, 12, 20, 28):
                                    drain()    # weave one projection unit
                                if len(pt_q) == 6:
                                    win = pt_q[:4]
                                    del pt_q[:4]
                                    consume4(win)
                            tail_win = list(pt_q)
                            pending_tail.append(
                                lambda w=tail_win, c=consume4: c(w))
                            pending_norm.append((h0, av, z_tot))
                    do_tail()   # last pair's final AV window
                    do_norm()   # last pair's normalization (PSUM-free)

            # ---------- Phase 4: output projection ----------
            MT = SC // P  # 4 query-row tiles
            with (
                tc.tile_pool(name="y_psum", bufs=8, space="PSUM") as y_psum,
                tc.tile_pool(name="y_sb", bufs=3) as ys_pool,
            ):
                NW = HID // 2 // 512  # 2 moving chunks of 512 per half
                for half in range(2):
                    ps = [[y_psum.tile([P, 512], FP, name="yp", tag="yp")
                           for _ in range(NW)] for _ in range(MT)]

                    def evac(m, half=half, ps=ps):
                        ysb = ys_pool.tile([P, HID // 2], BF)
                        # split the PSUM->SBUF eviction across the scalar and
                        # vector engines so half1's slots free up sooner
                        nc.vector.tensor_copy(ysb[:, 0:512], ps[m][0][:])
                        nc.scalar.activation(
                            ysb[:, 512:1024], ps[m][1][:],
                            mybir.ActivationFunctionType.Copy,
                        )
                        nc.sync.dma_start(
                            out=y[m * P:(m + 1) * P,
                                  half * (HID // 2):(half + 1) * (HID // 2)],
                            in_=ysb[:],
                        )

                    for k in range(KT):
                        if half == 0 and k < 2:
                            wot = wo_pre[k]
                        else:
                            wot = wo_pool.tile([P, HID // 2], BF)
                            nc.sync.dma_start(out=wot[:], in_=wo[half, k])
                        for m in range(MT):
                            for n in range(NW):
                                nc.tensor.matmul(
                                    ps[m][n][:],
                                    attT_sb[:, k, m * P:(m + 1) * P],
                                    wot[:, n * 512:(n + 1) * 512],
                                    start=(k == 0), stop=(k == KT - 1),
                                )
                            if k == KT - 1:
                                # evict each m-tile the moment its last
                                # accumulation retires
                                evac(m)
    # bacc lowering: splits multi-sem waits (HW allows 1 wait/instruction),
    # moves matmul waits onto LDWEIGHTS, register alloc.
    nc.compile()
    return nc


_CACHED = {}


def _prep_inputs(x, Wq, Wk, Wv, Wo):
    xs = np.ascontiguousarray(x.reshape(S, HID)).astype(np.float32)
    xT = np.ascontiguousarray(xs.T).astype(BF_NP)        # [HID, S]
    wqT = np.ascontiguousarray(Wq.T).astype(BF_NP)       # [HID, NH*D]
    # wq tiled: [P, NH*KT*D] with col ((o*KT)+h)*D+d = wqT[h*P+p, o*D+d]
    wq_t = np.empty((P, NH * KT * D), BF_NP)
    for o in range(NH):
        for h in range(KT):
            c0 = (o * KT + h) * D
            wq_t[:, c0:c0 + D] = wqT[h * P:(h + 1) * P, o * D:(o + 1) * D]
    wkT = np.ascontiguousarray(Wk.T).astype(BF_NP)       # [HID, NKV*D]
    wvT = np.ascontiguousarray(Wv.T).astype(BF_NP)
    woT = np.ascontiguousarray(Wo.T).astype(BF_NP)       # [HID(o), HID(m)]
    wo_t = np.empty((2, KT, P, HID // 2), BF_NP)
    for half in range(2):
        for k in range(KT):
            wo_t[half, k] = woT[k * P:(k + 1) * P,
                                half * (HID // 2):(half + 1) * (HID // 2)]
    in_maps = []
    for c in range(NC):
        in_maps.append({
            "xTc": np.ascontiguousarray(xT[:, c * SC:(c + 1) * SC]),
            "wq": wq_t, "wk": wkT, "wv": wvT, "wo": wo_t,
        })
    return in_maps


def run(x, Wq, Wk, Wv, Wo, trace=False):
    if "nc" not in _CACHED:
        _CACHED["nc"] = build_bass()
    nc = _CACHED["nc"]
    in_maps = _prep_inputs(x, Wq, Wk, Wv, Wo)
    res = run_bass_kernel_spmd(nc, in_maps, list(range(NC)), trace=trace)
    out = np.concatenate(
        [np.asarray(res.results[c]["y"]).astype(np.float32) for c in range(NC)],
        axis=0)
    return out.reshape(1, S, HID), res


def kernel(x, Wq, Wk, Wv, Wo):
    out, _ = run(np.asarray(x), np.asarray(Wq), np.asarray(Wk),
                 np.asarray(Wv), np.asarray(Wo))
    return out
